# revision 18
# baseline (speedup 1.0000x reference)
"""Self-contained Trainium2 Bass kernel for nn_BipartiteDataEncoder.
Accepts full inputs, shards across 8 NeuronCores internally.

Host-side preprocessing: sharding, permutations, edge schedules.

Terminology:
  - var nodes are dst of direction 'cv' (cons->var); cons nodes are dst of 'vc'.
  - Nodes are relabeled: node -> core (node % 8) -> degree-sorted rank within core.
    Padded shard sizes: SV=25088 (var, 196 blocks), SC=12544 (cons, 98 blocks).
  - Source tables are windowed so int16 local gather indices fit.
    var table: 8 equal windows of 25088 rows (core-pair major).
    cons table: 4 rank-chunk windows (rank ranges across all cores) so that
    chunked AllGathers of cons1 can fire mid-vc and cv can start early.
  - Edges of a core/direction are grouped by (dst block b, src window w), padded
    to whole 128-edge tiles; tile counts T[b,w] shared across cores (SPMD).
  - Each direction runs as two window-half passes (A/B) with per-block SBUF
    accumulators, so pass A can overlap the producer of the later windows.
"""
import numpy as np

NCONS, NVAR, NEDGE, EMB = 100_000, 200_000, 2_000_000, 64
NCORE = 8
BLK = 128
SV = 25_088   # var shard (padded), 196 blocks
SC = 12_544   # cons shard (padded), 98 blocks
VP, CP = SV * NCORE, SC * NCORE
WBOUND_V = [25_088 * i for i in range(9)]                  # 8 var windows
CBLKS = [25, 25, 24, 24]                                   # cons chunk blocks
CRANK = [b * BLK for b in CBLKS]                           # ranks per chunk
CBOUND_L = np.cumsum([0] + CRANK).tolist()                 # local rank bounds
WBOUND_C = np.cumsum([0] + [r * NCORE for r in CRANK]).tolist()  # table rows
CHUNK_END_BLOCKS = np.cumsum(CBLKS).tolist()               # [25,50,74,98]


def node_permutation(n_nodes, shard_size, deg):
    pid = np.empty(n_nodes, dtype=np.int64)
    for k in range(NCORE):
        nodes = np.arange(k, n_nodes, NCORE)
        order = np.argsort(-deg[nodes], kind="stable")
        pid[nodes[order]] = k * shard_size + np.arange(len(nodes))
    return pid


def gpid_of_cons(pid_c_arr):
    """Map canonical cons pid (core*SC + rank) -> chunked table row id."""
    core = pid_c_arr // SC
    rank = pid_c_arr % SC
    w = np.searchsorted(CBOUND_L, rank, side="right") - 1
    crk = np.asarray(CRANK)[w]
    return np.asarray(WBOUND_C)[w] + core * crk + (rank - np.asarray(CBOUND_L)[w])


def build_schedule(src_pid_of_edge, dst_pid_of_edge, shard_size, bounds,
                   break_blocks=()):
    """Tiled edge schedule for one direction. bounds: window row boundaries."""
    nblk = shard_size // BLK
    n_windows = len(bounds) - 1
    bnd = np.asarray(bounds)
    dst_core = dst_pid_of_edge // shard_size
    dst_loc = dst_pid_of_edge % shard_size
    b_of = dst_loc // BLK
    din = dst_loc % BLK
    w_of = np.searchsorted(bnd, src_pid_of_edge, side="right") - 1
    src_loc = (src_pid_of_edge - bnd[w_of]).astype(np.int64)
    assert src_loc.max() < 32768

    key_all = (dst_core * nblk + b_of) * n_windows + w_of
    cnt_flat = np.bincount(key_all, minlength=NCORE * nblk * n_windows)
    counts = cnt_flat.reshape(NCORE, nblk, n_windows)
    T = np.ceil(counts.max(axis=0) / BLK).astype(np.int64)      # [nblk, W]

    TILE_BUDGET = 104
    MAXB = 16
    tile_of = np.zeros((nblk, n_windows), dtype=np.int64)
    per_block = T.sum(axis=1)
    groups = []
    t = 0
    b = 0
    brk = set(break_blocks)
    while b < nblk:
        blocks = [b]
        tot = per_block[b]
        b += 1
        while (b < nblk and b not in brk and len(blocks) < MAXB
               and tot + per_block[b] <= TILE_BUDGET):
            tot += per_block[b]
            blocks.append(b)
            b += 1
        runs = []
        for w in range(n_windows):
            run_start = t
            for bb in blocks:
                tile_of[bb, w] = t
                t += T[bb, w]
            if t > run_start:
                runs.append((w, run_start, t - run_start))
        groups.append((blocks, runs))
    ntiles = t

    idx16 = np.zeros((NCORE, ntiles * BLK), dtype=np.int16)
    dstloc = np.full((NCORE, ntiles * BLK), -1.0, dtype=np.float32)
    order = np.lexsort((w_of, b_of, dst_core))
    sc, sb, sw = dst_core[order], b_of[order], w_of[order]
    ssrc, sdin = src_loc[order], din[order]
    key = (sc * nblk + sb) * n_windows + sw
    first = np.r_[True, key[1:] != key[:-1]]
    grp_start = np.maximum.accumulate(np.where(first, np.arange(len(key)), 0))
    rank = np.arange(len(key)) - grp_start
    pos = tile_of[sb, sw] * BLK + rank
    idx16[sc, pos] = ssrc.astype(np.int16)
    dstloc[sc, pos] = sdin.astype(np.float32)

    return dict(T=T, tile_of=tile_of, ntiles=ntiles, groups=groups,
                idx16=idx16, dstloc=dstloc, counts=counts)


def preprocess(inputs):
    inp = {k: np.asarray(v) for k, v in inputs.items()}
    row = inp["edge_index"][0].astype(np.int64)
    col = inp["edge_index"][1].astype(np.int64)

    deg_v = np.bincount(col, minlength=NVAR)
    deg_c = np.bincount(row, minlength=NCONS)
    pid_v = node_permutation(NVAR, SV, deg_v)
    pid_c = node_permutation(NCONS, SC, deg_c)
    row_p = pid_c[row]        # cons pid per edge (canonical core-major)
    col_p = pid_v[col]        # var pid per edge
    grow_p = gpid_of_cons(row_p)   # cons table row per edge (chunked layout)

    sched_cv = build_schedule(grow_p, col_p, SV, WBOUND_C)   # dst var
    sched_vc = build_schedule(col_p, row_p, SC, WBOUND_V,
                              break_blocks=CHUNK_END_BLOCKS[:-1])  # dst cons

    gpid_c = gpid_of_cons(pid_c)   # per cons NODE: table row

    def padT(x, pid, P):
        xt = np.zeros((P, x.shape[1]), dtype=np.float32)
        xt[pid] = x
        return np.ascontiguousarray(xt.T)

    cons_xT = padT(inp["cons_x"].astype(np.float32), pid_c, CP)    # own-shard
    cons_xTg = padT(inp["cons_x"].astype(np.float32), gpid_c, CP)  # table order
    var_xT = padT(inp["var_x"].astype(np.float32), pid_v, VP)
    breakT = padT(inp["break_indicator"].astype(np.float32), pid_v, VP)

    def fold(W1, b1, shift, scale):
        W1f = scale[:, None] * W1
        b1f = b1 + (shift * scale) @ W1
        return W1f.astype(np.float32), b1f.astype(np.float32)

    cW1f, cb1f = fold(inp["cons_W1"], inp["cons_b1"], inp["cons_pn_shift"], inp["cons_pn_scale"])
    vW1f, vb1f = fold(inp["var_W1"], inp["var_b1"], inp["var_pn_shift"], inp["var_pn_scale"])

    deg_v_p = np.zeros(VP, dtype=np.float32); deg_v_p[pid_v] = deg_v
    deg_c_p = np.zeros(CP, dtype=np.float32); deg_c_p[pid_c] = deg_c
    recip_v = (1.0 / np.maximum(deg_v_p, 1.0)).reshape(NCORE, SV)
    recip_c = (1.0 / np.maximum(deg_c_p, 1.0)).reshape(NCORE, SC)
    brk = inp["break_indicator"].astype(np.float64)[:, 0]
    bsum = np.zeros(CP, dtype=np.float64)
    np.add.at(bsum, row_p, brk[col])
    bsum_c = bsum.astype(np.float32).reshape(NCORE, SC)

    return dict(
        pid_v=pid_v, pid_c=pid_c,
        sched_cv=sched_cv, sched_vc=sched_vc,
        cons_xT=cons_xT, cons_xTg=cons_xTg, var_xT=var_xT, breakT=breakT,
        cW1f=cW1f, cb1f=cb1f, cW2=inp["cons_W2"].astype(np.float32), cb2=inp["cons_b2"].astype(np.float32),
        vW1f=vW1f, vb1f=vb1f, vW2=inp["var_W2"].astype(np.float32), vb2=inp["var_b2"].astype(np.float32),
        breakW=inp["break_W"].astype(np.float32),
        Wl_cv=inp["Wl_cv"].astype(np.float32), bl_cv=inp["bl_cv"].astype(np.float32),
        Wr_cv=inp["Wr_cv"].astype(np.float32),
        Wl_vc=inp["Wl_vc"].astype(np.float32), bl_vc=inp["bl_vc"].astype(np.float32),
        Wr_vc=inp["Wr_vc"].astype(np.float32),
        recip_v=recip_v, recip_c=recip_c, bsum_c=bsum_c,
    )


# ---- kernel builder ----
import contextlib
import numpy as np
import ml_dtypes
import concourse.bacc as bacc
import concourse.bass as bass
import concourse.mybir as mybir
import concourse.tile as tile
from concourse.masks import make_identity

F32 = mybir.dt.float32
BF16 = mybir.dt.bfloat16
I16 = mybir.dt.int16
RELU = mybir.ActivationFunctionType.Relu
COPY = mybir.ActivationFunctionType.Copy
ADD = mybir.AluOpType.add
MULT = mybir.AluOpType.mult
ISEQ = mybir.AluOpType.is_equal
STRIPE = 2048
OUT_W = SV // 2
GCHUNK = 16


def bf(x):
    return np.asarray(np.asarray(x, dtype=np.float32), dtype=ml_dtypes.bfloat16)


def build(P, phases="EVC"):
    nc = bacc.Bacc("TRN2", target_bir_lowering=False,
                   dynamic_dma_scratch_size=65536, num_swdge_queues=4)
    scv, svc = P["sched_cv"], P["sched_vc"]
    NT_CV, NT_VC = scv["ntiles"], svc["ntiles"]

    def inp(name, shape, dt):
        return nc.dram_tensor(name, shape, dt, kind="ExternalInput")

    cxTg = inp("cxTg", [6, CP], BF16)
    vxT = inp("vxT", [20, VP], BF16)
    o_vxT = inp("o_vxT", [20, SV], BF16)
    o_cxT = inp("o_cxT", [6, SC], BF16)
    o_brk = inp("o_brk", [1, SV], BF16)
    c_l1 = inp("c_l1", [6, 64], BF16)
    c_l2 = inp("c_l2", [65, 64], BF16)
    v_l1 = inp("v_l1", [20, 64], BF16)
    v_l2 = inp("v_l2", [65, 64], BF16)
    w_in = {}
    for nm in ("wl_vc", "wr_vc", "wl_cv0", "wr_cv0", "wl_cv1", "wr_cv1"):
        w_in[nm] = inp(nm, [64, 64], BF16)
    bl_in = {}
    for nm in ("bl_vc", "bl_cv0", "bl_cv1"):
        bl_in[nm] = inp(nm, [64, 1], F32)
    brkw_f = inp("brkw_f", [64, 1], F32)
    iota = inp("iota", [1, 128], BF16)
    recv = inp("recv", [1, SV], BF16)
    recc = inp("recc", [1, SC], BF16)
    bsum = inp("bsum", [1, SC], BF16)
    vc_idx = inp("vc_idx", [128, NT_VC * 8], I16)
    vc_dst = inp("vc_dst", [128, NT_VC], F32)
    cv_idx = inp("cv_idx", [128, NT_CV * 8], I16)
    cv_dst = inp("cv_dst", [128, NT_CV], F32)

    out = nc.dram_tensor("out", [128, OUT_W], BF16, kind="ExternalOutput")

    var_tabs = [nc.dram_tensor(f"var_tab{w}", [WBOUND_V[w + 1] - WBOUND_V[w], 128], BF16)
                for w in range(8)]
    cv_tabs = [nc.dram_tensor(f"cv_tab{w}", [WBOUND_C[w + 1] - WBOUND_C[w], 128], BF16)
               for w in range(4)]
    vroot = nc.dram_tensor("vroot", [64, SV], BF16)
    croot = nc.dram_tensor("croot", [64, SC], BF16)
    ag_in = nc.dram_tensor("ag_in", [SC, 64], BF16)
    ag_out = [nc.dram_tensor(f"ag_out{k}", [CRANK[k] * NCORE, 64], BF16,
                             addr_space="Shared") for k in range(4)]

    vwin_w = [[] for _ in range(len(WBOUND_V) - 1)]
    cwin_w = [[] for _ in range(len(WBOUND_C) - 1)]
    ag_writes_chunk = [[] for _ in range(4)]
    root_w = {'vroot': [], 'croot': []}
    coll_insts = [None] * 4

    with tile.TileContext(nc) as tc, contextlib.ExitStack() as stk:
        cpool = stk.enter_context(tc.tile_pool(name="consts", bufs=1))
        t_iota = cpool.tile([128, 128], BF16)
        nc.sync.dma_start(out=t_iota[:], in_=iota[0:1, :].partition_broadcast(128).squeeze(1))
        t_w = {}
        for nm, h in w_in.items():
            t_w[nm] = cpool.tile([64, 64], BF16, tag=nm, name="t_" + nm)
            nc.sync.dma_start(out=t_w[nm][:], in_=h[:])
        t_bl = {}
        for nm, h in bl_in.items():
            t_bl[nm] = cpool.tile([64, 1], F32, tag=nm, name="tb_" + nm)
            nc.sync.dma_start(out=t_bl[nm][:], in_=h[:])
        t_brkf = cpool.tile([64, 1], F32, tag="brkf")
        nc.sync.dma_start(out=t_brkf[:], in_=brkw_f[:])
        t_ident = cpool.tile([64, 64], BF16)
        make_identity(nc, t_ident[:])
        t_id128 = cpool.tile([128, 128], BF16)
        make_identity(nc, t_id128[:])
        t_l1c = cpool.tile([6, 64], BF16, tag="l1c")
        nc.sync.dma_start(out=t_l1c[:], in_=c_l1[:])
        t_l1v = cpool.tile([20, 64], BF16, tag="l1v")
        nc.sync.dma_start(out=t_l1v[:], in_=v_l1[:])
        t_l2c = cpool.tile([65, 64], BF16, tag="l2c")
        nc.sync.dma_start(out=t_l2c[:], in_=c_l2[:])
        t_l2v = cpool.tile([65, 64], BF16, tag="l2v")
        nc.sync.dma_start(out=t_l2v[:], in_=v_l2[:])

        h1_pool = stk.enter_context(tc.tile_pool(name="h1s", bufs=1))
        h1s = []
        for j in range(2):
            h1 = h1_pool.tile([65, STRIPE], BF16, tag=f"h1_{j}", name=f"h1_{j}")
            nc.vector.memset(h1[:], 1.0)
            h1s.append(h1)

        # persistent per-block accumulators
        accv_pool = stk.enter_context(tc.tile_pool(name="accv", bufs=1))
        accv = [accv_pool.tile([64, 128], BF16, tag=f"av{b}", name=f"av{b}") for b in range(SC // BLK)]

        def pass_tile_range(runs, wins):
            ts = [(s, s + n) for (w, s, n) in runs if w in wins]
            if not ts:
                return None
            return min(a for a, _ in ts), max(b for _, b in ts)

        # ---- prefetch vc pass-A aux (before embed; no deps, loads run early)
        VC_WINS_A = (0, 1, 2, 3)
        VC_WINS_B = (4, 5, 6, 7)
        CV_WINS_A = (0, 1)
        CV_WINS_B = (2, 3)
        vcA_aux = {}
        with nc.named_scope("prefetch"):
            axp = stk.enter_context(tc.tile_pool(name="vcA_aux", bufs=1))
            for gi, (blocks, runs) in enumerate(svc["groups"]):
                if gi >= 12:
                    break
                rng = pass_tile_range(runs, VC_WINS_A)
                if rng is None:
                    continue
                t0, t1 = rng
                idx_sb = axp.tile([128, (t1 - t0) * 8], I16, tag=f"ia{gi}")
                nc.scalar.dma_start(out=idx_sb[:], in_=vc_idx[:, t0 * 8:t1 * 8])
                vcA_aux[gi] = (idx_sb, t0, t1)

        # message-pass pools opened BEFORE embed so their SBUF/PSUM ranges are
        # disjoint from embed's pools — address reuse would add WAR deps that
        # serialize the first gathers behind the end of embed.
        gp = stk.enter_context(tc.tile_pool(name="gsb", bufs=8))
        ap_ = stk.enter_context(tc.tile_pool(name="aux", bufs=2))
        ohp = stk.enter_context(tc.tile_pool(name="ohp", bufs=6))
        sp = stk.enter_context(tc.tile_pool(name="sums", bufs=2, space="PSUM"))
        npp = stk.enter_context(tc.tile_pool(name="news", bufs=2, space="PSUM"))
        ep = stk.enter_context(tc.tile_pool(name="eout", bufs=3))
        stp = stk.enter_context(tc.tile_pool(name="stage", bufs=3))
        e_ep = stk.enter_context(tc.tile_pool(name="emb", bufs=2))
        e_pp = stk.enter_context(tc.tile_pool(name="emb_ps", bufs=2, space="PSUM"))
        e_op = stk.enter_context(tc.tile_pool(name="emb_out", bufs=3))

        # ---------- Phase E ----------
        def embed_tables(xT_h, l1_t, l2_t, ncols, tables, win_writes, wbound):
            kin = xT_h.shape[0]
            if True:
                ep, pp, op = e_ep, e_pp, e_op
                for si, s0 in enumerate(range(0, ncols, STRIPE)):
                    sw = min(STRIPE, ncols - s0)
                    xs = ep.tile([kin, STRIPE], BF16, tag=f"xs{kin}")
                    nc.sync.dma_start(out=xs[:, :sw], in_=xT_h[:, s0:s0 + sw])
                    h1 = h1s[si % 2]
                    for c0 in range(0, sw, 512):
                        ps = pp.tile([64, 512], F32, tag="ps1")
                        nc.tensor.matmul(ps[:], lhsT=l1_t[:], rhs=xs[:, c0:c0 + 512],
                                         start=True, stop=True)
                        nc.scalar.activation(h1[0:64, c0:c0 + 512], ps[:], RELU)
                    for c0 in range(0, sw, 512):
                        ps2 = pp.tile([128, 256], F32, tag="ps2")
                        for j in range(4):
                            cc = c0 + j * 128
                            nc.tensor.matmul(ps2[:, j * 64:(j + 1) * 64],
                                             lhsT=h1[:, cc:cc + 128], rhs=l2_t[:],
                                             start=True, stop=True)
                        ot = op.tile([128, 256], BF16, tag="ot")
                        nc.scalar.activation(ot[:], ps2[:], RELU)
                        r0 = s0 + c0
                        w = int(np.searchsorted(wbound, r0, side="right") - 1)
                        lr = r0 - wbound[w]
                        di = nc.sync.dma_start(
                            out=tables[w][lr:lr + 512, 0:64].rearrange("(a p) f -> p a f", p=128),
                            in_=ot[:].rearrange("p (a f) -> p a f", a=4))
                        win_writes[w].append(di.ins)

        def embed_own(xT_h, l1_t, l2_t, ncols, root, brk, wlist):
            kin = xT_h.shape[0]
            if True:
                ep, pp, op = e_ep, e_pp, e_op
                for si, s0 in enumerate(range(0, ncols, STRIPE)):
                    sw = min(STRIPE, ncols - s0)
                    xs = ep.tile([kin, STRIPE], BF16, tag=f"xs{kin}")
                    nc.sync.dma_start(out=xs[:, :sw], in_=xT_h[:, s0:s0 + sw])
                    h1 = h1s[si % 2]
                    brt = None
                    if brk is not None:
                        brt = ep.tile([64, STRIPE], BF16, tag="brt")
                        nc.sync.dma_start(out=brt[:, :sw],
                                          in_=brk[0:1, s0:s0 + sw].partition_broadcast(64).squeeze(1))
                    for c0 in range(0, sw, 512):
                        ps = pp.tile([64, 512], F32, tag="ps1")
                        nc.tensor.matmul(ps[:], lhsT=l1_t[:], rhs=xs[:, c0:c0 + 512],
                                         start=True, stop=True)
                        nc.scalar.activation(h1[0:64, c0:c0 + 512], ps[:], RELU)
                    for c0 in range(0, sw, 512):
                        cw = min(512, ncols - (s0 + c0))
                        ps2 = pp.tile([64, 512], F32, tag="ps2")
                        nc.tensor.matmul(ps2[:], lhsT=l2_t[:], rhs=h1[:, c0:c0 + 512],
                                         start=True, stop=True)
                        ot = op.tile([64, 512], BF16, tag="ot")
                        nc.scalar.activation(ot[:], ps2[:], RELU)
                        if brk is not None:
                            tmp = op.tile([64, 512], BF16, tag="tmp")
                            nc.vector.tensor_scalar(
                                out=tmp[:], in0=brt[:, c0:c0 + 512],
                                scalar1=t_brkf[:], scalar2=None, op0=MULT)
                            nc.vector.tensor_tensor(out=ot[:], in0=ot[:], in1=tmp[:], op=ADD)
                        wlist.append(nc.sync.dma_start(out=root[:, s0 + c0:s0 + c0 + cw], in_=ot[:, :cw]).ins)

        with nc.named_scope("embed"):
            embed_tables(vxT, t_l1v, t_l2v, VP, var_tabs, vwin_w, WBOUND_V)
            embed_tables(cxTg, t_l1c, t_l2c, CP, cv_tabs, cwin_w, WBOUND_C)
            embed_own(o_vxT, t_l1v, t_l2v, SV, vroot, o_brk, root_w['vroot'])
            embed_own(o_cxT, t_l1c, t_l2c, SC, croot, None, root_w['croot'])

        # ---------- message pass ----------
        qrr = [0]

        def msg_pass_half(sched, wins, src_tab, bounds, idx_h, dst_h, win_deps,
                          is_cv, is_final, acc, gp, ap_, ohp, sp, npp, ep, stp,
                          prefetched=None, chunk_hook=None):
            """One window-half pass over all groups of a direction."""
            for gi, (blocks, runs) in enumerate(sched["groups"]):
                if prefetched is not None and gi in prefetched:
                    idx_sb, g_t0, g_t1 = prefetched[gi]
                    dst_sb = ap_.tile([128, g_t1 - g_t0], F32, tag="dst")
                    nc.scalar.dma_start(out=dst_sb[:], in_=dst_h[:, g_t0:g_t1])
                else:
                    rng = pass_tile_range(runs, wins)
                    if rng is None:
                        if chunk_hook:
                            chunk_hook(blocks[-1])
                        continue
                    g_t0, g_t1 = rng
                    gw = g_t1 - g_t0
                    idx_sb = ap_.tile([128, gw * 8], I16, tag="idx")
                    nc.scalar.dma_start(out=idx_sb[:], in_=idx_h[:, g_t0 * 8:g_t1 * 8])
                    dst_sb = ap_.tile([128, gw], F32, tag="dst")
                    nc.scalar.dma_start(out=dst_sb[:], in_=dst_h[:, g_t0:g_t1])
                b0 = blocks[0]
                nb = len(blocks)
                rec_sb = bs_sb = None
                if is_final:
                    rec_h = recv if is_cv else recc
                    rec_sb = ap_.tile([64, 128 * 16], BF16, tag="rec")
                    nc.scalar.dma_start(
                        out=rec_sb[:, :nb * 128],
                        in_=rec_h[0:1, b0 * 128:(b0 + nb) * 128].partition_broadcast(64).squeeze(1))
                    if not is_cv:
                        bs_sb = ap_.tile([64, 128 * 16], BF16, tag="bs")
                        nc.scalar.dma_start(
                            out=bs_sb[:, :nb * 128],
                            in_=bsum[0:1, b0 * 128:(b0 + nb) * 128].partition_broadcast(64).squeeze(1))
                chunk_tiles = {}
                for (w, ts, n) in runs:
                    if w not in wins:
                        continue
                    s = ts
                    while s < ts + n:
                        m = min(GCHUNK, ts + n - s)
                        g = gp.tile([128, GCHUNK, 128], BF16, tag="g")
                        gi_ = nc.gpsimd.dma_gather(
                            out_ap=g[:, :m, :],
                            in_ap=src_tab[w][:, :],
                            idxs_ap=idx_sb[:, (s - g_t0) * 8:(s - g_t0 + m) * 8],
                            num_idxs=m * 128, num_idxs_reg=m * 128,
                            elem_size=128, single_packet=False,
                            queue_num=qrr[0] % 4)
                        qrr[0] += 1
                        for dep in win_deps[w]:
                            tile.add_dep_helper(gi_.ins, dep, reason="tab->gather")
                        for t in range(s, s + m):
                            chunk_tiles[t] = (g, s)
                        s += m
                for b in blocks:
                    ntl = int(sum(sched["T"][b, w] for w in wins))
                    ps = sp.tile([128 if is_cv else 64, 128], F32, tag="ps")
                    done = 0
                    if is_final and is_cv and acc is not None:
                        # re-inject pass-A partial sums via identity matmul
                        nc.tensor.matmul(ps[:], lhsT=t_id128[:], rhs=acc[b][:],
                                         start=True, stop=(ntl == 0))
                        done += 1
                    elif ntl == 0:
                        if chunk_hook:
                            chunk_hook(b)
                        continue
                    tot = ntl + done
                    for w in wins:
                        t0 = int(sched["tile_of"][b, w])
                        for t in range(t0, t0 + int(sched["T"][b, w])):
                            g, base = chunk_tiles[t]
                            tl = t - g_t0
                            oh = ohp.tile([128, 128], BF16, tag="oh")
                            nc.vector.tensor_scalar(
                                out=oh[:], in0=t_iota[:],
                                scalar1=dst_sb[:, tl:tl + 1],
                                scalar2=None, op0=ISEQ)
                            lhs = g[:, t - base, :] if is_cv else g[:, t - base, 0:64]
                            done += 1
                            nc.tensor.matmul(ps[:], lhsT=lhs, rhs=oh[:],
                                             start=(done == 1), stop=(done == tot))
                    if not is_final:
                        # stash partial sums in SBUF accumulator
                        nc.scalar.activation(acc[b][:], ps[0:acc[b].shape[0], :], COPY)
                        continue
                    c0, c1_ = b * 128, (b + 1) * 128
                    rsl = rec_sb[:, (b - b0) * 128:(b - b0 + 1) * 128]
                    if is_cv:
                        meanA = ep.tile([64, 128], BF16, tag="meanA")
                        nc.vector.tensor_tensor(out=meanA[:], in0=ps[0:64, :], in1=rsl, op=MULT)
                        meanB = ep.tile([64, 128], BF16, tag="meanB")
                        nc.vector.tensor_tensor(out=meanB[:], in0=ps[64:128, :], in1=rsl, op=MULT)
                        xr = ep.tile([64, 128], BF16, tag="xr")
                        xri = nc.sync.dma_start(out=xr[:], in_=vroot[:, c0:c1_])
                        for dep in root_w['vroot']:
                            tile.add_dep_helper(xri.ins, dep, reason="root->xr")
                        np1 = npp.tile([64, 128], F32, tag="np")
                        nc.tensor.matmul(np1[:], lhsT=t_w["wl_cv0"][:], rhs=meanA[:],
                                         start=True, stop=False)
                        nc.tensor.matmul(np1[:], lhsT=t_w["wr_cv0"][:], rhs=xr[:],
                                         start=False, stop=True)
                        v1 = ep.tile([64, 128], BF16, tag="v1")
                        nc.scalar.activation(v1[:], np1[:], RELU, bias=t_bl["bl_cv0"][:])
                        np2 = npp.tile([64, 128], F32, tag="np")
                        nc.tensor.matmul(np2[:], lhsT=t_w["wl_cv1"][:], rhs=meanB[:],
                                         start=True, stop=False)
                        nc.tensor.matmul(np2[:], lhsT=t_w["wr_cv1"][:], rhs=v1[:],
                                         start=False, stop=True)
                        vo = stp.tile([64, 128], BF16, tag="vo")
                        nc.scalar.activation(vo[:], np2[:], RELU, bias=t_bl["bl_cv1"][:])
                        nc.sync.dma_start(
                            out=out[(b % 2) * 64:(b % 2) * 64 + 64,
                                    (b // 2) * 128:(b // 2) * 128 + 128],
                            in_=vo[:])
                    else:
                        s1 = ep.tile([64, 128], F32, tag="s1")
                        nc.vector.tensor_tensor(out=s1[:], in0=ps[:], in1=acc[b][:], op=ADD)
                        tmp = ep.tile([64, 128], F32, tag="tmp")
                        nc.vector.tensor_scalar(
                            out=tmp[:], in0=bs_sb[:, (b - b0) * 128:(b - b0 + 1) * 128],
                            scalar1=t_brkf[:], scalar2=None, op0=MULT)
                        s2 = ep.tile([64, 128], F32, tag="s2")
                        nc.vector.tensor_tensor(out=s2[:], in0=s1[:], in1=tmp[:], op=ADD)
                        mean = ep.tile([64, 128], BF16, tag="mean")
                        nc.vector.tensor_tensor(out=mean[:], in0=s2[:], in1=rsl, op=MULT)
                        xr = ep.tile([64, 128], BF16, tag="xr")
                        xri = nc.sync.dma_start(out=xr[:], in_=croot[:, c0:c1_])
                        for dep in root_w['croot']:
                            tile.add_dep_helper(xri.ins, dep, reason="root->xr")
                        np1 = npp.tile([64, 128], F32, tag="np")
                        nc.tensor.matmul(np1[:], lhsT=t_w["wl_vc"][:], rhs=mean[:],
                                         start=True, stop=False)
                        nc.tensor.matmul(np1[:], lhsT=t_w["wr_vc"][:], rhs=xr[:],
                                         start=False, stop=True)
                        c1t = ep.tile([64, 128], BF16, tag="c1t")
                        nc.scalar.activation(c1t[:], np1[:], RELU, bias=t_bl["bl_vc"][:])
                        tp = npp.tile([128, 64], BF16, tag="np")
                        nc.tensor.transpose(out=tp[:], in_=c1t[:], identity=t_ident[:])
                        nm = stp.tile([128, 64], BF16, tag="nm")
                        nc.scalar.activation(nm[:], tp[:], COPY)
                        k = int(np.searchsorted(CHUNK_END_BLOCKS, b, side="right"))
                        ag_writes_chunk[k].append(
                            nc.sync.dma_start(out=ag_in[c0:c1_, :], in_=nm[:]).ins)
                    if chunk_hook:
                        chunk_hook(b)

        def emit_coll(k):
            lo, hi = CBOUND_L[k], CBOUND_L[k + 1]
            coll = nc.gpsimd.collective_compute(
                "AllGather", mybir.AluOpType.bypass,
                ins=[ag_in[lo:hi, :]], outs=[ag_out[k][:]],
                replica_groups=[list(range(NCORE))])
            for wi in ag_writes_chunk[k]:
                tile.add_dep_helper(coll.ins, wi, reason="agin->coll")
            coll_insts[k] = coll.ins

        def emit_spread(k):
            nrows = CRANK[k] * NCORE
            with tc.tile_pool(name=f"spread{k}", bufs=2) as spp:
                for r0 in range(0, nrows, 2048):
                    m = min(2048, nrows - r0)
                    st = spp.tile([128, 16, 64], BF16, tag="st")
                    rd = nc.sync.dma_start(
                        out=st[:, :m // 128, :],
                        in_=ag_out[k][r0:r0 + m, :].rearrange("(a p) f -> p a f", p=128))
                    tile.add_dep_helper(rd.ins, coll_insts[k], reason="coll->spread")
                    di = nc.sync.dma_start(
                        out=cv_tabs[k][r0:r0 + m, 64:128].rearrange("(a p) f -> p a f", p=128),
                        in_=st[:, :m // 128, :])
                    cwin_w[k].append(di.ins)

        done_chunks = set()

        def vc_chunk_hook(b):
            k = int(np.searchsorted(CHUNK_END_BLOCKS, b, side="right"))
            if b == CHUNK_END_BLOCKS[k] - 1 and k not in done_chunks:
                done_chunks.add(k)
                emit_coll(k)
                emit_spread(k)

        with nc.named_scope("vc"):
            msg_pass_half(svc, VC_WINS_A, var_tabs, WBOUND_V, vc_idx, vc_dst,
                          vwin_w, is_cv=False, is_final=False, acc=accv,
                          gp=gp, ap_=ap_, ohp=ohp, sp=sp, npp=npp, ep=ep,
                          stp=stp, prefetched=vcA_aux)
            msg_pass_half(svc, VC_WINS_B, var_tabs, WBOUND_V, vc_idx, vc_dst,
                          vwin_w, is_cv=False, is_final=True, acc=accv,
                          gp=gp, ap_=ap_, ohp=ohp, sp=sp, npp=npp, ep=ep,
                          stp=stp, chunk_hook=vc_chunk_hook)
        if "C" in phases:
            with nc.named_scope("cv"):
                msg_pass_half(scv, (0, 1, 2, 3), cv_tabs, WBOUND_C, cv_idx, cv_dst,
                              cwin_w, is_cv=True, is_final=True, acc=None,
                              gp=gp, ap_=ap_, ohp=ohp, sp=sp, npp=npp, ep=ep,
                              stp=stp)

    nc.finalize()
    return nc


def wrap_idx(flat):
    w = flat.reshape(-1, 16).T
    return np.ascontiguousarray(np.tile(w, (8, 1)))


def colmaj(flat):
    return np.ascontiguousarray(flat.reshape(-1, 128).T)


def in_map(P, core):
    s = P
    return {
        "cxTg": bf(np.vstack([s["cons_xTg"], np.ones((1, CP), np.float32)])),
        "vxT": bf(np.vstack([s["var_xT"], np.ones((1, VP), np.float32)])),
        "o_vxT": bf(np.vstack([s["var_xT"][:, core * SV:(core + 1) * SV],
                               np.ones((1, SV), np.float32)])),
        "o_cxT": bf(np.vstack([s["cons_xT"][:, core * SC:(core + 1) * SC],
                               np.ones((1, SC), np.float32)])),
        "o_brk": bf(s["breakT"][:, core * SV:(core + 1) * SV]),
        "c_l1": bf(np.vstack([s["cW1f"], s["cb1f"][None, :]])),
        "c_l2": bf(np.vstack([s["cW2"], s["cb2"][None, :]])),
        "v_l1": bf(np.vstack([s["vW1f"], s["vb1f"][None, :]])),
        "v_l2": bf(np.vstack([s["vW2"], s["vb2"][None, :]])),
        "wl_vc": bf(s["Wl_vc"][0]), "wr_vc": bf(s["Wr_vc"][0]),
        "wl_cv0": bf(s["Wl_cv"][0]), "wr_cv0": bf(s["Wr_cv"][0]),
        "wl_cv1": bf(s["Wl_cv"][1]), "wr_cv1": bf(s["Wr_cv"][1]),
        "bl_vc": np.ascontiguousarray(s["bl_vc"][0][:, None], dtype=np.float32),
        "bl_cv0": np.ascontiguousarray(s["bl_cv"][0][:, None], dtype=np.float32),
        "bl_cv1": np.ascontiguousarray(s["bl_cv"][1][:, None], dtype=np.float32),
        "brkw_f": np.ascontiguousarray(s["breakW"][0][:, None], dtype=np.float32),
        "iota": bf(np.arange(128, dtype=np.float32)[None, :]),
        "recv": bf(s["recip_v"][core][None, :]),
        "recc": bf(s["recip_c"][core][None, :]),
        "bsum": bf(s["bsum_c"][core][None, :]),
        "vc_idx": wrap_idx(s["sched_vc"]["idx16"][core]),
        "vc_dst": colmaj(s["sched_vc"]["dstloc"][core]),
        "cv_idx": wrap_idx(s["sched_cv"]["idx16"][core]),
        "cv_dst": colmaj(s["sched_cv"]["dstloc"][core]),
    }


def unpack_out(outs_per_core, pid_v):
    var2T = np.zeros((64, NCORE * SV), dtype=np.float32)
    for k, o in enumerate(outs_per_core):
        o = np.asarray(o, dtype=np.float32).reshape(128, OUT_W // 128, 128)
        base = k * SV
        for half in range(2):
            blocks = o[half * 64:(half + 1) * 64]
            npair = blocks.shape[1]
            idxs = (np.arange(npair) * 2 + half) * 128
            for i, c in enumerate(idxs):
                var2T[:, base + c:base + c + 128] = blocks[:, i, :]
    return var2T.T[pid_v]


# ---------------- top-level kernel entry ----------------
_CACHE = {}


def kernel(**inputs):
    import numpy as _np
    key = "k"
    if key not in _CACHE:
        P = preprocess(inputs)
        nc = build(P, phases="EVC")
        _CACHE[key] = (P, nc)
    P, nc = _CACHE[key]
    from concourse.bass_utils import run_bass_kernel_spmd
    in_maps = [in_map(P, k) for k in range(NCORE)]
    res = run_bass_kernel_spmd(nc, in_maps, core_ids=list(range(NCORE)))
    outs = [res.results[k]["out"] for k in range(NCORE)]
    return unpack_out(outs, P["pid_v"]).astype(_np.float32)
